# revision 41
# baseline (speedup 1.0000x reference)
"""Trainium2 Bass kernel for nn_NSRLossV2 (8-core SPMD).

Math (reference, fp32):
    a  = x @ W1 + b1            [B, H]
    h  = relu(a)
    z  = h @ W2 + b2            [B, C]
    mse    = mean((z - onehot(y))^2)
    margin = sum(relu(1 - z_y + z) * (1 - onehot)) / B
    grads  = ((a > 0) * W2[:, y].T) @ W1.T        [B, D]
    w_l1   = sum_d |grads|
    R      = w_l1 * EPS / (|z_y| + 1e-8)
    nsr    = BETA * mean(log(1 + R))
    acc    = mean(argmax(z) == y)
    loss   = mse + (margin + nsr) * acc

Sharding: model-parallel over H (each core owns a 512-wide slice of W1's
columns) with the batch split into 2 pipeline chunks of 256.  mm1 runs in
bf16 (fills the ~40-60us collective-firmware wake-up window and keeps the
z/argmax path accurate); mm3 runs in fp8e4 DoubleRow (per-tensor scales
folded into EPS host-side).  Per chunk, G = (a>0)*W2[:,y] (exact in fp8
because W2[:,y] is pre-quantized) is AllGathered together with the fp32
z-partials in one contiguous payload; chunk 0's AllGather overlaps chunk
1's mm1, chunk 1's AllGather overlaps chunk 0's mm3.  The payload buffers
are declared uint8: fp8-typed collective buffers get fp8-NaN byte patterns
canonicalized on multi-hop mesh paths, which corrupts bitcast-packed fp32
data (diagnosed empirically; ranks 0-1 arrive intact, ranks 2-7 mangled).
Per-sample |grads| L1 partials are AllGathered once at the end and reduced
on-chip.  A tiny warmup AllGather fires first thing on the (otherwise
empty) gpsimd queue to start the collective firmware's wake-up as early
as possible.
"""

import os
import functools

import numpy as np
import ml_dtypes

import concourse.bass as bass
import concourse.bacc as bacc
import concourse.mybir as mybir
import concourse.tile as tile
from concourse.bass_utils import run_bass_kernel_spmd

NCORES = 8
B, D, H, C = 512, 4096, 4096, 5
HC = H // NCORES          # per-core H slice (512)
DC = D // NCORES          # per-core D slice (512)
P = 128
KT = D // P               # 32 k-tiles over D (mm1) / over H (mm3)
KG = 4                    # k-tiles per grouped DMA
NG = KT // KG             # 8 grouped loads
MT = HC // P              # 4 m-tiles over the local H slice
BT = B // P               # 4 b-tiles over the batch
NCH = 2                   # batch pipeline chunks
BCS = [256, 256]          # chunk widths (batch samples; measured better than
                          # 384/128 — smaller AG0 beats the shorter tail chunk)
TCS = [w // P for w in BCS]          # b-tiles per chunk (3, 1)
OFFS = [0, BCS[0]]                   # sample offset per chunk
TOFF = [0, TCS[0]]                   # b-tile offset per chunk
BETA, EPS = 0.4, 0.05
GB_C = [HC * w for w in BCS]         # G block bytes per chunk (fp8)
ZB_C = [P * t * C * 4 for t in TCS]  # z-partial block bytes (fp32)
PAY_C = [g + z for g, z in zip(GB_C, ZB_C)]
DEBUG = False

F32 = mybir.dt.float32
BF16 = mybir.dt.bfloat16
F8 = mybir.dt.float8e4
U8 = mybir.dt.uint8
E4NP = ml_dtypes.float8_e4m3

LAST_RESULTS = None  # BassKernelResults of the most recent HW run


def ts(i, n):
    return slice(i * n, (i + 1) * n)


def build_kernel():
    nc = bacc.Bacc(
        "TRN2",
        target_bir_lowering=False,
        debug=False,
        enable_asserts=False,
        num_devices=NCORES,
    )

    # ---- I/O (per-core shards prepared on host) -------------------------
    xtg0 = nc.dram_tensor("xtg0", [NG, P, KG * BCS[0]], BF16, kind="ExternalInput")
    xtg1 = nc.dram_tensor("xtg1", [NG, P, KG * BCS[1]], BF16, kind="ExternalInput")
    xtg = [xtg0, xtg1]
    w1g = nc.dram_tensor("w1g", [NG, P, KG * HC], BF16, kind="ExternalInput")
    wrg = nc.dram_tensor("wrg", [NG, P, KG * DC], F8, kind="ExternalInput")
    w2sel = nc.dram_tensor("w2sel", [HC, B], F8, kind="ExternalInput")
    w2c = nc.dram_tensor("w2c", [HC, C], F32, kind="ExternalInput")
    b1c = nc.dram_tensor("b1c", [MT, P], F32, kind="ExternalInput")
    yoh = nc.dram_tensor("yoh", [P, BT * C], F32, kind="ExternalInput")
    yohi = nc.dram_tensor("yohi", [P, BT * C], F32, kind="ExternalInput")
    b2bc = nc.dram_tensor("b2bc", [P, BT * C], F32, kind="ExternalInput")
    cin = nc.dram_tensor("cin", [P, 1], F32, kind="ExternalInput")  # EPS'
    out = nc.dram_tensor("out", [1, 1], F32, kind="ExternalOutput")
    dbg = nc.dram_tensor("dbg", [P, 240], F32, kind="ExternalOutput") if DEBUG \
        else None

    rg = [list(range(NCORES))]

    with tile.TileContext(nc) as tc:
        with (
            tc.tile_pool(name="dram", bufs=1, space="DRAM") as dpool,
            tc.tile_pool(name="xtp", bufs=3) as xtp,
            tc.tile_pool(name="w1p", bufs=NG) as w1p,
            tc.tile_pool(name="wrp", bufs=NG) as wrp,
            tc.tile_pool(name="gfp", bufs=8) as gfp,
            tc.tile_pool(name="hp", bufs=2 * MT) as hp,
            tc.tile_pool(name="w2sp", bufs=MT) as w2sp,
            tc.tile_pool(name="glp", bufs=NCH) as glp,
            tc.tile_pool(name="resident", bufs=1) as res,
            tc.tile_pool(name="psA", bufs=4, space="PSUM") as psA,
            tc.tile_pool(name="psG", bufs=TCS[0], space="PSUM") as psG,
            tc.tile_pool(name="psZ", bufs=1, space="PSUM") as psZ,
        ):
            # ---- collective bounce buffers in DRAM -----------------------
            gin = [dpool.tile([PAY_C[ch]], U8, name=f"gin{ch}") for ch in range(NCH)]
            gout = [
                dpool.tile([NCORES * PAY_C[ch]], U8, name=f"gout{ch}",
                           addr_space="Shared")
                for ch in range(NCH)
            ]
            w_in = dpool.tile([P * BT], F32, name="w_in")
            w_out = dpool.tile([NCORES * P * BT], F32, name="w_out",
                               addr_space="Shared")
            warm_in = dpool.tile([1, 8], F32, name="warm_in")
            warm_out = dpool.tile([NCORES, 8], F32, name="warm_out",
                                  addr_space="Shared")

            # warmup collective, first thing on the gpsimd queue: starts the
            # collective firmware's ~55us wake-up immediately.  Input is an
            # unwritten scratch buffer (result unused; AllGather = pure copy).
            nc.gpsimd.collective_compute(
                "AllGather", mybir.AluOpType.bypass,
                replica_groups=rg, ins=[warm_in.opt()], outs=[warm_out.opt()],
            )

            # ---- early constants (scalar queue, ahead of the w1 stream) --
            ones_col = res.tile([P, 1], F32, name="ones_col")
            nc.vector.memset(ones_col[:], 1.0)
            eps8 = res.tile([P, 1], F32, name="eps8")
            nc.vector.memset(eps8[:], 1e-8)
            b1t = res.tile([P, MT], F32, name="b1t")
            for m in range(MT):
                nc.gpsimd.dma_start(b1t[:, m : m + 1], b1c[m])
            w2t = res.tile([P, MT * C], F32, name="w2t")  # [128, (m,c)]
            for m in range(MT):
                nc.gpsimd.dma_start(
                    w2t[:, ts(m, C)], w2c.rearrange("(m p) c -> m p c", p=P)[m]
                )
            w2s_t = []
            for m in range(MT):
                w2s_m = w2sp.tile([P, B], F8, name="w2s_m", tag="w2s")
                nc.gpsimd.dma_start(w2s_m[:], w2sel[ts(m, P), :])
                w2s_t.append(w2s_m)

            S = res.tile([P, 16], F32, name="S")
            zt = [res.tile([P, TCS[ch] * C], F32, name=f"zt{ch}")
                  for ch in range(NCH)]
            rec = res.tile([P, BT], F32, name="rec")
            zy = res.tile([P, BT], F32, name="zy")
            wl1_p = res.tile([P, BT], F32, name="wl1_p")

            # G block stored with rows permuted as (hh, p, two) so mm3's gf
            # loads read one contiguous run per partition:
            #   flat addr = ((hh*128 + p)*2 + two)*BC + b
            #   holds G[h_local = hh*256 + two*128 + p, b]
            gin_g = [
                gin[ch].bitcast(F8)[0 : GB_C[ch]].rearrange(
                    "(hh p two b) -> hh p two b", hh=2, p=P, two=2
                )
                for ch in range(NCH)
            ]
            gin_z = [
                gin[ch].bitcast(F32)[GB_C[ch] // 4 : PAY_C[ch] // 4].rearrange(
                    "(p q) -> p q", q=TCS[ch] * C
                )
                for ch in range(NCH)
            ]

            DR = mybir.MatmulPerfMode.DoubleRow
            h_t = [[None] * MT for _ in range(NCH)]
            w1p_tiles = []

            # ================= per-chunk: mm1 + mm2 + G + AG =============
            for ch in range(NCH):
                BC = BCS[ch]
                ps_a = [
                    psA.tile([P, BC], F32, name=f"ps_a{ch}{m}", tag="psA")
                    for m in range(MT)
                ]
                for g in range(NG):
                    xt_g = xtp.tile([P, KG * BC], BF16, name="xt_g", tag="xt")
                    nc.sync.dma_start(xt_g[:], xtg[ch][g])
                    xt3 = xt_g[:].rearrange("p (i b) -> p i b", i=KG)
                    if ch == 0:
                        # split the 4MB w1 stream across both HWDGE queues
                        w1_r = w1p.tile([P, KG * HC], BF16, name=f"w1_{g}",
                                        tag="w1")
                        eng = nc.scalar if g % 2 == 0 else nc.sync
                        eng.dma_start(w1_r[:], w1g[g])
                        w1p_tiles.append(w1_r)
                    w13 = w1p_tiles[g][:].rearrange("p (i h) -> p i h", i=KG)
                    for i in range(KG):
                        k = g * KG + i
                        for m in range(MT):
                            nc.tensor.matmul(
                                ps_a[m][:],
                                w13[:, i, ts(m, P)],
                                xt3[:, i, :],
                                start=(k == 0),
                                stop=(k == KT - 1),
                            )

                # h = relu(a + b1) on DVE
                for m in range(MT):
                    h_m = hp.tile([P, BC], F32, name=f"h{ch}{m}", tag="h")
                    nc.vector.tensor_scalar(
                        h_m[:], ps_a[m][:], b1t[:, m : m + 1], 0.0,
                        op0=mybir.AluOpType.add, op1=mybir.AluOpType.max,
                    )
                    h_t[ch][m] = h_m

                # mm2: z-partial [b, c] on PE (fp32, tiny)
                for t in range(TCS[ch]):
                    ps_z = psZ.tile([P, C], F32, name="ps_z", tag="psZ")
                    for m in range(MT):
                        nc.tensor.matmul(
                            ps_z[:], h_t[ch][m][:, ts(t, P)], w2t[:, ts(m, C)],
                            start=(m == 0), stop=(m == MT - 1),
                        )
                    nc.vector.tensor_copy(zt[ch][:, ts(t, C)], ps_z[:])

                # G = (h > 0) * W2sel'  (exact fp8), pack G + z into payload
                g_l = glp.tile([P, MT * BC], F8, name=f"gl{ch}", tag="gl")
                for m in range(MT):
                    nc.vector.scalar_tensor_tensor(
                        g_l[:, ts(m, BC)], h_t[ch][m][:], 0.0,
                        w2s_t[m][:, OFFS[ch] : OFFS[ch] + BC],
                        op0=mybir.AluOpType.is_gt, op1=mybir.AluOpType.mult,
                    )
                    nc.sync.dma_start(
                        gin_g[ch][m // 2, :, m % 2, :], g_l[:, ts(m, BC)]
                    )
                nc.sync.dma_start(gin_z[ch], zt[ch][:])

                nc.gpsimd.collective_compute(
                    "AllGather", mybir.AluOpType.bypass,
                    replica_groups=rg, ins=[gin[ch].opt()], outs=[gout[ch].opt()],
                )

            # ---- late constants + mm3 weight stream (scalar queue) ------
            epsc = res.tile([P, 1], F32, name="epsc")
            nc.scalar.dma_start(epsc[:], cin[:])
            b2bt = res.tile([P, BT * C], F32, name="b2bt")
            nc.scalar.dma_start(b2bt[:], b2bc[:])
            yoht = res.tile([P, BT * C], F32, name="yoht")
            yohit = res.tile([P, BT * C], F32, name="yohit")
            nc.scalar.dma_start(yoht[:], yoh[:])
            nc.scalar.dma_start(yohit[:], yohi[:])
            wr_tiles = []
            for g in range(NG):
                wr_g = wrp.tile([P, KG * DC], F8, name="wr_g", tag="wr")
                nc.scalar.dma_start(wr_g[:], wrg[g])
                wr_tiles.append(wr_g[:].rearrange("p (i d) -> p i d", i=KG))

            # ================= mm3: grads = G @ W1_c.T ====================
            for ch in range(NCH):
                BC = BCS[ch]
                ps_g = [
                    psG.tile([P, DC], F32, name=f"ps_g{ch}{t}", tag="psG")
                    for t in range(TCS[ch])
                ]
                for kp in range(KT // 2):
                    r, hh = divmod(kp, 2)
                    base = r * PAY_C[ch] + hh * 2 * P * BC
                    gf = gfp.tile([P, 2 * BC], F8, name="gf", tag="gf")
                    if kp == 0:
                        # split the first tile by b-halves across both queues:
                        # the t=0 matmul only needs the first half, so PE can
                        # start ~0.4us sooner after the collective completes
                        src3 = gout[ch].bitcast(F8)[
                            base : base + 2 * P * BC
                        ].rearrange("(p two b) -> p two b", p=P, two=2)
                        gf3d = gf[:].rearrange("p (two b) -> p two b", two=2)
                        nc.scalar.dma_start(gf3d[:, :, 0:P], src3[:, :, 0:P])
                        nc.sync.dma_start(gf3d[:, :, P:BC], src3[:, :, P:BC])
                    else:
                        eng = nc.scalar if kp < 2 else nc.sync
                        eng.dma_start(
                            gf[:],
                            gout[ch].bitcast(F8)[
                                base : base + 2 * P * BC
                            ].rearrange("(p tb) -> p tb", p=P),
                        )
                    gf3 = gf[:].rearrange("p (two b) -> p two b", two=2)
                    g_idx, i_idx = divmod(kp, 2)
                    for t in range(TCS[ch]):
                        nc.tensor.matmul(
                            ps_g[t][:],
                            gf3[:, :, ts(t, P)],
                            wr_tiles[g_idx][:, 2 * i_idx : 2 * i_idx + 2, :],
                            start=(kp == 0),
                            stop=(kp == KT // 2 - 1),
                            perf_mode=DR,
                        )
                for t in range(TCS[ch]):
                    gt = TOFF[ch] + t
                    nc.vector.reduce_sum(
                        wl1_p[:, gt : gt + 1], ps_g[t][:],
                        axis=mybir.AxisListType.X, apply_absolute_value=True,
                    )

            # ---- per-chunk z phase: unpack + sum + loss stats -----------
            zacc_t, zf_t = [], []
            for ch in range(NCH):
                W = TCS[ch] * C
                o = TOFF[ch]
                gout_f = gout[ch].bitcast(F32)
                zacc = res.tile([P, W * NCORES], F32, name=f"zacc{ch}")
                zacc_t.append(zacc)
                for r in range(NCORES):
                    src = gout_f[
                        (r * PAY_C[ch] + GB_C[ch]) // 4 : (r + 1) * PAY_C[ch] // 4
                    ].rearrange("(p q) -> p q", q=W)
                    nc.scalar.dma_start(zacc[:, ts(r, W)], src)
                zp4 = res.tile([P, W * 4], F32, name=f"zp4{ch}")
                nc.vector.tensor_add(zp4[:], zacc[:, : W * 4], zacc[:, W * 4 :])
                zp2 = res.tile([P, W * 2], F32, name=f"zp2{ch}")
                nc.vector.tensor_add(zp2[:], zp4[:, : W * 2], zp4[:, W * 2 :])
                zp1 = res.tile([P, W], F32, name=f"zp1{ch}")
                nc.vector.tensor_add(zp1[:], zp2[:, :W], zp2[:, W:])
                zf_c = res.tile([P, W], F32, name=f"zf{ch}")
                nc.vector.tensor_add(zf_c[:], zp1[:], b2bt[:, o * C : o * C + W])
                zf_t.append(zf_c)

                # mse partial
                dz = res.tile([P, W], F32, name=f"dz{ch}")
                nc.vector.tensor_sub(dz[:], zf_c[:], yoht[:, o * C : o * C + W])
                dz2 = res.tile([P, W], F32, name=f"dz2{ch}")
                nc.vector.tensor_mul(dz2[:], dz[:], dz[:])
                nc.vector.reduce_sum(
                    S[:, o : o + TCS[ch]],
                    dz2[:].rearrange("p (t c) -> p t c", c=C),
                    axis=mybir.AxisListType.X,
                )
                # z_y, correctness, margin
                zyh = res.tile([P, W], F32, name=f"zyh{ch}")
                nc.vector.tensor_mul(zyh[:], zf_c[:], yoht[:, o * C : o * C + W])
                nc.vector.reduce_sum(
                    zy[:, o : o + TCS[ch]],
                    zyh[:].rearrange("p (t c) -> p t c", c=C),
                    axis=mybir.AxisListType.X,
                )
                zmax = res.tile([P, TCS[ch]], F32, name=f"zmax{ch}")
                nc.vector.reduce_max(
                    zmax[:], zf_c[:].rearrange("p (t c) -> p t c", c=C),
                    axis=mybir.AxisListType.X,
                )
                nc.vector.tensor_tensor(
                    S[:, 8 + o : 8 + o + TCS[ch]],
                    zy[:, o : o + TCS[ch]], zmax[:],
                    op=mybir.AluOpType.is_ge,
                )
                omz = res.tile([P, TCS[ch]], F32, name=f"omz{ch}")
                nc.vector.tensor_scalar(
                    omz[:], zy[:, o : o + TCS[ch]], -1.0, 1.0,
                    op0=mybir.AluOpType.mult, op1=mybir.AluOpType.add,
                )
                mg = res.tile([P, W], F32, name=f"mg{ch}")
                for t in range(TCS[ch]):
                    nc.scalar.activation(
                        mg[:, ts(t, C)], zf_c[:, ts(t, C)],
                        mybir.ActivationFunctionType.Relu,
                        bias=omz[:, t : t + 1],
                    )
                mgm = res.tile([P, W], F32, name=f"mgm{ch}")
                nc.vector.tensor_mul(mgm[:], mg[:], yohit[:, o * C : o * C + W])
                nc.vector.reduce_sum(
                    S[:, 4 + o : 4 + o + TCS[ch]],
                    mgm[:].rearrange("p (t c) -> p t c", c=C),
                    axis=mybir.AxisListType.X,
                )
                den = res.tile([P, TCS[ch]], F32, name=f"den{ch}")
                nc.scalar.activation(
                    den[:], zy[:, o : o + TCS[ch]],
                    mybir.ActivationFunctionType.Abs, bias=eps8[:, 0:1],
                )
                rec0 = res.tile([P, TCS[ch]], F32, name=f"rec0{ch}")
                nc.vector.reciprocal(rec0[:], den[:])
                # fold EPS' in here so the post-collective tail saves an op
                nc.vector.tensor_scalar(
                    rec[:, o : o + TCS[ch]], rec0[:], epsc[:, 0:1], 0.0,
                    op0=mybir.AluOpType.mult, op1=mybir.AluOpType.add,
                )

            # ---- wl1 AllGather + on-chip rank sum ------------------------
            nc.sync.dma_start(
                w_in.rearrange("(p t) -> p t", t=BT), wl1_p[:]
            )
            nc.gpsimd.collective_compute(
                "AllGather", mybir.AluOpType.bypass,
                replica_groups=rg, ins=[w_in.opt()], outs=[w_out.opt()],
            )

            # early final part: mse/margin/corr sums (overlaps the AG)
            ps12 = psZ.tile([1, 12], F32, name="ps12", tag="psZ")
            nc.tensor.matmul(ps12[:], ones_col[:], S[:, 0:12], start=True, stop=True)
            tots = res.tile([1, 3], F32, name="tots")
            nc.vector.reduce_sum(
                tots[:], ps12[:].rearrange("p (s t) -> p s t", t=BT),
                axis=mybir.AxisListType.X,
            )
            partA = res.tile([1, 1], F32, name="partA")
            nc.vector.tensor_scalar(
                partA[:], tots[:, 0:1], 1.0 / (B * C), 0.0,
                op0=mybir.AluOpType.mult, op1=mybir.AluOpType.add,
            )
            coef = res.tile([1, 1], F32, name="coef")
            nc.vector.tensor_scalar(
                coef[:], tots[:, 2:3], 1.0 / (B * B), 0.0,
                op0=mybir.AluOpType.mult, op1=mybir.AluOpType.add,
            )

            wlacc = res.tile([P, NCORES * BT], F32, name="wlacc")
            w_out_v = w_out.rearrange("(r p t) -> p r t", p=P, t=BT)
            hr = NCORES // 2
            nc.scalar.dma_start(
                wlacc[:, : hr * BT].rearrange("p (r t) -> p r t", t=BT),
                w_out_v[:, 0:hr, :],
            )
            nc.sync.dma_start(
                wlacc[:, hr * BT :].rearrange("p (r t) -> p r t", t=BT),
                w_out_v[:, hr:NCORES, :],
            )
            wl1 = res.tile([P, BT], F32, name="wl1")
            nc.vector.reduce_sum(
                wl1[:], wlacc[:].rearrange("p (r t) -> p t r", t=BT),
                axis=mybir.AxisListType.X,
            )

            # nsr per-sample: log(1 + wl1 * (EPS' * rec))
            rt2 = res.tile([P, BT], F32, name="rt2")
            nc.vector.tensor_mul(rt2[:], wl1[:], rec[:])
            nc.scalar.activation(
                S[:, 12:16], rt2[:], mybir.ActivationFunctionType.Ln, bias=1.0
            )
            ps3 = psZ.tile([1, 4], F32, name="ps3", tag="psZ")
            nc.tensor.matmul(ps3[:], ones_col[:], S[:, 12:16], start=True, stop=True)
            nsr_s = res.tile([1, 1], F32, name="nsr_s")
            nc.vector.reduce_sum(nsr_s[:], ps3[:], axis=mybir.AxisListType.X)

            # loss = mse/(B*C) + (margin_sum + BETA*nsr_sum) * corr_sum/(B*B)
            t_a = res.tile([1, 1], F32, name="t_a")
            nc.vector.scalar_tensor_tensor(
                t_a[:], nsr_s[:], BETA, tots[:, 1:2],
                op0=mybir.AluOpType.mult, op1=mybir.AluOpType.add,
            )
            t_b = res.tile([1, 1], F32, name="t_b")
            nc.vector.tensor_mul(t_b[:], t_a[:], coef[:])
            t_g = res.tile([1, 1], F32, name="t_g")
            nc.vector.tensor_add(t_g[:], t_b[:], partA[:])
            nc.sync.dma_start(out[:], t_g[:])

            if DEBUG:
                dbg_sb = res.tile([P, 240], F32, name="dbg_sb")
                nc.vector.memset(dbg_sb[:], 0.0)
                nc.vector.tensor_copy(dbg_sb[:, 0:16], S[:])
                nc.vector.tensor_copy(dbg_sb[:, 16:20], zy[:])
                nc.vector.tensor_copy(dbg_sb[:, 20:24], wl1[:])
                nc.vector.tensor_copy(dbg_sb[:, 24:28], rec[:])
                for ch in range(NCH):
                    W = TCS[ch] * C
                    nc.vector.tensor_copy(dbg_sb[:, 28 + 20 * ch : 28 + 20 * ch + W],
                                          zt[ch][:])
                    n = min(W * NCORES, 80)
                    nc.vector.tensor_copy(dbg_sb[:, 68 + 80 * ch : 68 + 80 * ch + n],
                                          zacc_t[ch][:, 0:n])
                    nc.vector.tensor_copy(dbg_sb[:, 228 + 5 * ch : 233 + 5 * ch],
                                          zf_t[ch][:, 0:5])
                nc.scalar.dma_start(dbg[:], dbg_sb[:])

    nc.compile()
    return nc


def _pack_ktiles(arr, group=KG):
    """[K*128, N] row-major -> [K/group, 128, group*N] so each grouped DMA
    reads group*N contiguous bytes per partition row."""
    K = arr.shape[0] // P
    N = arr.shape[1]
    return np.ascontiguousarray(
        arr.reshape(K // group, group, P, N).transpose(0, 2, 1, 3).reshape(
            K // group, P, group * N
        )
    )


def _q8(v, s):
    return np.clip(v * s, -240.0, 240.0).astype(E4NP)


def prep_inputs(x, y, W1, b1, W2, b2):
    """Host-side shard + quantization + layout prep."""
    x = np.asarray(x, dtype=np.float32)
    y = np.asarray(y).astype(np.int64)
    W1 = np.asarray(W1, dtype=np.float32)
    b1 = np.asarray(b1, dtype=np.float32)
    W2 = np.asarray(W2, dtype=np.float32)
    b2 = np.asarray(b2, dtype=np.float32)
    bf = ml_dtypes.bfloat16

    sw1 = np.float32(240.0) / np.float32(np.abs(W1).max())
    w2sel_full = W2[:, y]                                   # [H, B]
    sg = np.float32(240.0) / np.float32(np.abs(w2sel_full).max())
    eps_p = np.float32(EPS / (float(sg) * float(sw1)))

    xt = np.ascontiguousarray(x.T).astype(bf)               # [D, B] bf16
    xtg_c = [
        _pack_ktiles(np.ascontiguousarray(xt[:, OFFS[ch] : OFFS[ch] + BCS[ch]]))
        for ch in range(NCH)
    ]
    w2sel8_full = _q8(w2sel_full, sg)                       # [H, B] fp8
    yoh = np.zeros((P, BT * C), np.float32)
    for t in range(BT):
        for p in range(P):
            yoh[p, t * C + int(y[t * P + p])] = 1.0
    yohi = (1.0 - yoh).astype(np.float32)
    b2bc = np.ascontiguousarray(np.tile(b2.reshape(1, C), (P, BT)).astype(np.float32))
    cin = np.full((P, 1), eps_p, np.float32)

    in_maps = []
    for c in range(NCORES):
        hs = slice(c * HC, (c + 1) * HC)
        ds = slice(c * DC, (c + 1) * DC)
        in_maps.append({
            "xtg0": xtg_c[0],
            "xtg1": xtg_c[1],
            "w1g": _pack_ktiles(W1[:, hs].astype(bf)),
            "wrg": _pack_ktiles(_q8(np.ascontiguousarray(W1[ds, :].T), sw1)),
            "w2sel": np.ascontiguousarray(w2sel8_full[hs, :]),
            "w2c": np.ascontiguousarray(W2[hs, :]),
            "b1c": np.ascontiguousarray(b1[hs].reshape(MT, P)),
            "yoh": yoh,
            "yohi": yohi,
            "b2bc": b2bc,
            "cin": cin,
        })
    return in_maps


@functools.lru_cache(maxsize=1)
def get_nc():
    return build_kernel()


def kernel(x, y, W1, b1, W2, b2):
    global LAST_RESULTS
    nc = get_nc()
    in_maps = prep_inputs(x, y, W1, b1, W2, b2)

    if os.environ.get("BASSK_SIM"):
        from concourse.bass_interp import MultiCoreSim
        sim = MultiCoreSim(
            nc, num_cores=NCORES, require_finite=False, require_nnan=False
        )
        for c in range(NCORES):
            for k, v in in_maps[c].items():
                sim.cores[c].tensor(k)[:] = v
        sim.simulate(check_with_hw=False)
        res = np.array(sim.cores[0].tensor("out"))
        if DEBUG:
            kernel.last_dbg = np.array(sim.cores[0].tensor("dbg"))
    else:
        r = run_bass_kernel_spmd(
            nc,
            in_maps,
            core_ids=list(range(NCORES)),
            trace=bool(os.environ.get("BASSK_TRACE")),
        )
        LAST_RESULTS = r
        res = r.results[0]["out"]
        if DEBUG:
            kernel.last_dbg = r.results[0]["dbg"]

    return np.float32(res.reshape(())).reshape(())
